# revision 21
# baseline (speedup 1.0000x reference)
"""Trainium2 Bass kernel for nn_AttentionLayer (B=64, S=512, F=256), 8 cores.

Reference computation (per batch b):
    scores = x1 @ Wq + x2 @ Wk          # [S, S]
    a = softmax(tanh(scores), axis=-1)   # softmax over u
    a2 = a @ Wv                          # [S, S]
    out = a2 * x1                        # elementwise
    out = out * rsqrt(max(sum_s out^2, eps))   # l2-normalize over axis s

Strategy: pure data parallelism -- 8 batches per core, weights replicated.
Everything is computed in a TRANSPOSED layout ([t-or-u partitions, s free]).

Design notes (informed by HW traces; v5 of the schedule):
  * Stage A consumes a BF16 copy of x1 (all weights bf16 too), so every
    A-feed is only 0.75MB/batch and all stage-A matmuls run at bf16
    cadence (~216ns vs ~236 for f32r moving).  The f32 x1 stream needed
    by the epilogue (dtype-pure q = y*x1 on DVE) trails ~1.5 windows
    behind on the wire -- it never gates the PE.
  * The startup DMA wire saturates at ~320GB/s total across the three
    rings (sync/scalar/gpsimd); everything is emitted in consumption
    order so the critical prefix (b0 strips, then per-batch bf16 feeds)
    leads.  Batch 0 runs k-outer so strips are consumed on arrival.
  * Stage C consumes the UNNORMALIZED expz; 1/denominator folds into the
    epilogue w-pass, so no PE work waits on the rowsum->recip chain.
  * The rowsum uses a ones-BLOCK stationary so the matmul writes the
    denominator replicated across all 128 partitions.
  * The tile scheduler is greedy-by-readiness per engine (emission order
    is only a tie-break priority): the drain emits b7's rowsum/recip
    right after stage_a(b7) so the recip outranks b6's q-chain on the
    DVE the moment the rowsum lands; the PE fills the wait with C(b6).
  * Finalize groups (b0,b1),(b2,b3),(b4,b5),(b6),b7: each group's ACT
    sqrts adjacent; Square lives in BOTH activation-table sets so the
    pre-drain swap to the sqrt set also covers the drain's squares+sqrt.
  * Drain: b6+b7 squares all on ACT (idle post-exp) keeping the DVE
    clear for the q/w chains; norms split GpSimd/DVE; output leaves in
    quarter-DMAs on alternating rings as tiles normalize.
  * All DRAM tensors partition-major; output bf16, upcast on host.
"""

import sys

sys.path.insert(0, "/opt/trn_rl_repo")

import numpy as np
import ml_dtypes

import concourse.bass as bass
import concourse.tile as tile
from concourse import bacc, mybir
from concourse.bass_utils import run_bass_kernel_spmd

B, S, F = 64, 512, 256
N_CORES = 8
BPC = B // N_CORES  # batches per core
P = 128
KT1 = S // P  # 4 k-tiles over t (x1/Wq contraction)
KT2 = F // P  # 2 k-tiles over f (x2/Wk contraction)
NT = S // P  # 4 m-tiles over u (stage A) / t (stage C)
EPS = 1e-12

F32 = mybir.dt.float32
BF16 = mybir.dt.bfloat16
AF = mybir.ActivationFunctionType
ALU = mybir.AluOpType

BFNP = ml_dtypes.bfloat16

last_results = None  # test harness introspection


def build_nc(reps=1, bpc=BPC):
    nc = bacc.Bacc(
        "TRN2", target_bir_lowering=False, debug=False, num_devices=N_CORES
    )
    # Partition-major packed tensors: [.., P, ktiles, S].
    x1b = nc.declare_dram_parameter("x1b", [bpc, P, KT1, S], BF16, isOutput=False)
    x1r = nc.declare_dram_parameter("x1r", [bpc, P, KT1, S], F32, isOutput=False)
    x2t = nc.declare_dram_parameter("x2t", [bpc, P, KT2, S], BF16, isOutput=False)
    wqb = nc.declare_dram_parameter("wqb", [P, KT1, S], BF16, isOutput=False)
    wk = nc.declare_dram_parameter("wk", [P, KT2, S], BF16, isOutput=False)
    wv = nc.declare_dram_parameter("wv", [P, NT, S], BF16, isOutput=False)
    out = nc.declare_dram_parameter("out", [bpc, P, NT, S], BF16, isOutput=True)

    batches = [bb for _ in range(reps) for bb in range(bpc)]
    nb = len(batches)

    with tile.TileContext(nc) as tc:
        with (
            tc.tile_pool(name="singles", bufs=1) as singles,
            tc.tile_pool(name="xin", bufs=1) as xin,
            tc.tile_pool(name="work", bufs=2) as work,
            tc.tile_pool(name="small", bufs=2) as small,
            tc.tile_pool(name="outp", bufs=2) as outp,
            tc.tile_pool(name="psA", bufs=2, space="PSUM") as psA,
            tc.tile_pool(name="psY", bufs=3, space="PSUM") as psY,
            tc.tile_pool(name="psR", bufs=1, space="PSUM") as psR,
        ):
            ones_blk = singles.tile([P, P], BF16)
            nc.vector.memset(ones_blk, 1.0)
            eps_t = singles.tile([P, 1], F32)
            nc.vector.memset(eps_t, EPS)

            def dma_ab(b):
                """bf16 A-feed for batch b (0.75MB): x1 bf16 + x2."""
                tb = xin.tile([P, KT1, S], BF16, tag="x1b", bufs=3)
                nc.sync.dma_start(out=tb, in_=x1b.ap()[b])
                t2 = xin.tile([P, KT2, S], BF16, tag="x2", bufs=3)
                nc.sync.dma_start(out=t2, in_=x2t.ap()[b])
                return tb, t2

            def dma_r(b):
                """f32 x1 for batch b's epilogue (1MB), halves on two
                rings; needed ~1.5 windows after emission."""
                tr = xin.tile([P, KT1, S], F32, tag="x1r", bufs=3)
                nc.scalar.dma_start(out=tr[:, 0:2, :], in_=x1r.ap()[b, :, 0:2, :])
                nc.sync.dma_start(out=tr[:, 2:4, :], in_=x1r.ap()[b, :, 2:4, :])
                return tr

            # Startup, in wire-consumption order: b0 strips (k-outer
            # consumes on arrival), b1's A-feed, wv, b0's f32 x1.
            wqb_t = singles.tile([P, KT1, S], BF16, tag="wqb")
            x1b0_t = xin.tile([P, KT1, S], BF16, tag="x1b", bufs=3)
            wk_t = singles.tile([P, KT2, S], BF16, tag="wk")
            x2_first = xin.tile([P, KT2, S], BF16, tag="x2", bufs=3)
            # kt0 as single-k strips so the first matmul fires ~2us sooner
            # (half-rate 1KB lines, but first-byte latency wins at the top)
            nc.sync.dma_start(out=wqb_t[:, 0:1, :], in_=wqb.ap()[:, 0:1, :])
            nc.scalar.dma_start(out=x1b0_t[:, 0:1, :], in_=x1b.ap()[0, :, 0:1, :])
            nc.sync.dma_start(out=wqb_t[:, 1:2, :], in_=wqb.ap()[:, 1:2, :])
            nc.scalar.dma_start(out=x1b0_t[:, 1:2, :], in_=x1b.ap()[0, :, 1:2, :])
            nc.sync.dma_start(out=wk_t, in_=wk.ap())
            nc.sync.dma_start(out=wqb_t[:, 2:4, :], in_=wqb.ap()[:, 2:4, :])
            nc.scalar.dma_start(out=x1b0_t[:, 2:4, :], in_=x1b.ap()[0, :, 2:4, :])
            nc.scalar.dma_start(out=x2_first, in_=x2t.ap()[0])
            ab_tiles = {0: (x1b0_t, x2_first)}
            if nb > 1:
                ab_tiles[1] = dma_ab(1)
            wv_t = singles.tile([P, NT, S], BF16, tag="wv")
            nc.scalar.dma_start(out=wv_t, in_=wv.ap())
            r_tiles = {0: dma_r(0)}

            def stage_a_b0():
                """k-outer stage A for batch 0: strips consumed as they
                land; both u-pair PSUM tiles accumulate simultaneously."""
                sc01 = psA.tile([P, 2, S], F32, tag="scores")
                sc23 = psA.tile([P, 2, S], F32, tag="scores")
                scs = (sc01, sc23)
                for kt in range(KT1):
                    for half in range(2):
                        for j in range(2):
                            ut = half * 2 + j
                            us = slice(ut * P, (ut + 1) * P)
                            nc.tensor.matmul(
                                scs[half][:, j, :],
                                wqb_t[:, kt, us],
                                x1b0_t[:, kt, :],
                                start=(kt == 0),
                                stop=False,
                            )
                for kf in range(KT2):
                    for half in range(2):
                        for j in range(2):
                            ut = half * 2 + j
                            us = slice(ut * P, (ut + 1) * P)
                            nc.tensor.matmul(
                                scs[half][:, j, :],
                                wk_t[:, kf, us],
                                x2_first[:, kf, :],
                                start=False,
                                stop=(kf == KT2 - 1),
                            )
                expz = work.tile([P, NT, S], BF16, tag="expz", bufs=3)
                for half in range(2):
                    tanh_t = work.tile([P, 2, S], F32, tag="tanh")
                    nc.scalar.activation(out=tanh_t, in_=scs[half], func=AF.Tanh)
                    nc.scalar.activation(
                        out=expz[:, half * 2 : half * 2 + 2, :],
                        in_=tanh_t,
                        func=AF.Exp,
                    )
                return expz

            def stage_a(b, x1_sb, x2_sb, mid_cb=None):
                """scores matmuls in u-tile pairs sharing one 2-bank PSUM
                tile, tanh+exp over pairs.  mid_cb (if set) is emitted
                between the two pair-halves (the previous batch's rowsum
                overlaps this batch's remaining matmuls)."""
                expz = work.tile([P, NT, S], BF16, tag="expz", bufs=3)
                for half in range(NT // 2):
                    sc = psA.tile([P, 2, S], F32, tag="scores")
                    for j in range(2):
                        ut = half * 2 + j
                        us = slice(ut * P, (ut + 1) * P)
                        prods = [
                            (wqb_t[:, kt, us], x1_sb[:, kt, :]) for kt in range(KT1)
                        ] + [(wk_t[:, kt, us], x2_sb[:, kt, :]) for kt in range(KT2)]
                        for pi, (l_ap, r_ap) in enumerate(prods):
                            nc.tensor.matmul(
                                sc[:, j, :],
                                l_ap,
                                r_ap,
                                start=(pi == 0),
                                stop=(pi == len(prods) - 1),
                            )
                    tanh_t = work.tile([P, 2, S], F32, tag="tanh")
                    nc.scalar.activation(out=tanh_t, in_=sc, func=AF.Tanh)
                    nc.scalar.activation(
                        out=expz[:, half * 2 : half * 2 + 2, :],
                        in_=tanh_t,
                        func=AF.Exp,
                    )
                    if half == 0 and mid_cb is not None:
                        mid_cb()
                return expz

            def stage_b(b, expz):
                """softmax denominator (ones-block rowsum -> broadcast
                form) + reciprocal; 1/D folds into the epilogue w-pass."""
                rs = psR.tile([P, S], F32, tag="rowsum")
                for ut in range(NT):
                    nc.tensor.matmul(
                        rs,
                        ones_blk,
                        expz[:, ut, :],
                        start=(ut == 0),
                        stop=(ut == NT - 1),
                    )
                rbc = small.tile([P, S], F32, tag="rbc")
                nc.vector.reciprocal_approx_fast(out=rbc, in_=rs)
                return rbc

            def stage_c(b, x1f_sb, expz, rbc, sq_act_all=False, tiles=None,
                        opened=None):
                """Y matmuls on raw expz; epilogue q=y*x1 -> w=q*rbc (f32 on
                DVE); square-accum split ACT/DVE (or all-ACT when the DVE
                is the drain-critical chain).  `tiles` selects a subset of
                t-tiles (the drain interleaves C(b6) into A(b7) halves);
                `opened` carries (w_sb, sumsq) across split calls."""
                if opened is None:
                    w_sb = outp.tile([P, NT, S], F32, tag="w", bufs=4)
                    sumsq = small.tile([P, NT], F32, tag="sumsq", bufs=5)
                else:
                    w_sb, sumsq = opened
                for tt in (range(NT) if tiles is None else tiles):
                    y = psY.tile([P, S], F32, tag="y")
                    for ut in range(NT):
                        nc.tensor.matmul(
                            y,
                            wv_t[:, ut, tt * P : (tt + 1) * P],
                            expz[:, ut, :],
                            start=(ut == 0),
                            stop=(ut == NT - 1),
                        )
                    q_t = small.tile([P, S], F32, tag="q")
                    w_t = w_sb[:, tt, :]
                    nc.vector.tensor_tensor(
                        out=q_t, in0=y, in1=x1f_sb[:, tt, :], op=ALU.mult
                    )
                    nc.vector.tensor_tensor(out=w_t, in0=q_t, in1=rbc, op=ALU.mult)
                    if tt >= 2 and not sq_act_all:
                        scr = small.tile([P, S], F32, tag="scr")
                        nc.vector.scalar_tensor_tensor(
                            out=scr,
                            in0=w_t,
                            scalar=1.0,
                            in1=w_t,
                            op0=ALU.mult,
                            op1=ALU.mult,
                            accum_out=sumsq[:, tt : tt + 1],
                        )
                    else:
                        scr = small.tile([P, S], BF16, tag="scrb")
                        nc.scalar.activation(
                            out=scr,
                            in_=w_t,
                            func=AF.Square,
                            accum_out=sumsq[:, tt : tt + 1],
                        )
                return w_sb, sumsq

            def stage_fin_group(fins, gpsimd_only=False, quarters=False):
                """Finalize a group: ACT sqrts adjacent (one sqrt-table
                epoch per group), norms split GpSimd/DVE (all-GpSimd for
                the pre-drain flush, keeping the DVE free)."""
                rsqs = []
                for (b, w_sb, sumsq) in fins:
                    rsq = small.tile([P, NT], F32, tag="rsq", bufs=5)
                    nc.scalar.activation(
                        out=rsq, in_=sumsq, func=AF.Sqrt, bias=eps_t
                    )
                    rsqs.append(rsq)
                for (b, w_sb, sumsq), rsq in zip(fins, rsqs):
                    ob = outp.tile([P, NT, S], BF16, tag="ob", bufs=4)
                    if gpsimd_only:
                        for tt in range(NT):
                            nc.gpsimd.normalize_recip(
                                out_ap=ob[:, tt, :],
                                in_ap=w_sb[:, tt, :],
                                denom_ap=rsq[:, tt : tt + 1],
                            )
                            if quarters and tt % 2 == 1:
                                # pre-drain halves ride sync/scalar so the
                                # gpsimd ring is empty at teardown (its
                                # DRAIN otherwise spins ~5us on late DMAs)
                                q_eng = nc.sync if tt == 1 else nc.scalar
                                q_eng.dma_start(
                                    out=out.ap()[b, :, tt - 1 : tt + 1, :],
                                    in_=ob[:, tt - 1 : tt + 1, :],
                                )
                    else:
                        vv = small.tile([P, NT], F32, tag="vv", bufs=4)
                        nc.vector.reciprocal_approx_fast(out=vv, in_=rsq)
                        for tt in range(NT):
                            if tt < 2:
                                nc.gpsimd.normalize_recip(
                                    out_ap=ob[:, tt, :],
                                    in_ap=w_sb[:, tt, :],
                                    denom_ap=rsq[:, tt : tt + 1],
                                )
                            else:
                                nc.vector.tensor_scalar_mul(
                                    ob[:, tt, :],
                                    w_sb[:, tt, :],
                                    vv[:, tt : tt + 1],
                                )
                    if not (gpsimd_only and quarters):
                        # scalar ring: keeps the gpsimd DMA ring empty all
                        # run so its teardown DRAIN is ~0.1us, not ~3.5us
                        nc.scalar.dma_start(out=out.ap()[b], in_=ob)

            def stage_fin_last(b, w_sb, sumsq):
                """Drain finalize: norms split GpSimd/DVE, quarter-DMAs on
                alternating rings as tiles complete."""
                rsq = small.tile([P, NT], F32, tag="rsq", bufs=5)
                nc.scalar.activation(out=rsq, in_=sumsq, func=AF.Sqrt, bias=eps_t)
                vv = small.tile([P, NT], F32, tag="vv", bufs=4)
                nc.vector.reciprocal_approx_fast(out=vv, in_=rsq)
                ob = outp.tile([P, NT, S], BF16, tag="ob", bufs=4)
                for tt in range(NT):
                    if tt % 2 == 0:
                        nc.gpsimd.normalize_recip(
                            out_ap=ob[:, tt, :],
                            in_ap=w_sb[:, tt, :],
                            denom_ap=rsq[:, tt : tt + 1],
                        )
                        nc.sync.dma_start(
                            out=out.ap()[b, :, tt : tt + 1, :],
                            in_=ob[:, tt : tt + 1, :],
                        )
                    else:
                        nc.vector.tensor_scalar_mul(
                            ob[:, tt, :], w_sb[:, tt, :], vv[:, tt : tt + 1]
                        )
                        nc.scalar.dma_start(
                            out=out.ap()[b, :, tt : tt + 1, :],
                            in_=ob[:, tt : tt + 1, :],
                        )

            flush_at = (
                {2: 2, 4: 2, 6: 2, nb - 1: 1}
                if nb >= 7
                else {nb - 1: nb - 1}
            )

            pending = None  # (b, x1r_tile, expz) awaiting stages B+C
            fins = []  # (b, w_sb, sumsq) awaiting finalize
            rbc_last = None
            for i, b in enumerate(batches):
                if i + 2 < nb:
                    ab_tiles[i + 2] = dma_ab(i + 2)
                if i + 1 < nb:
                    r_tiles[i + 1] = dma_r(i + 1)
                prev = pending
                hold = {}

                def mid_cb():
                    hold["rbc"] = stage_b(prev[0], prev[2])
                    if i == nb - 1:
                        # drain: interleave C(b6) INTO A(b7) -- tt0 between
                        # the halves, tt1-3 right after -- so b6's q/w DVE
                        # chain starts ~4us before the PE finishes.  b6's
                        # squares all ride ACT (idle post-exp) keeping the
                        # DVE clear.  PSUM peak stays at 7/8 banks.
                        hold["opened"] = stage_c(
                            prev[0], prev[1], prev[2], hold["rbc"],
                            sq_act_all=True, tiles=[0],
                        )

                if i == 0:
                    expz = stage_a_b0()
                else:
                    xb, x2c = ab_tiles.pop(i)
                    expz = stage_a(b, xb, x2c, mid_cb)
                if prev is not None:
                    if i == nb - 1:
                        c_res = stage_c(
                            prev[0], prev[1], prev[2], hold["rbc"],
                            sq_act_all=True, tiles=[1, 2, 3],
                            opened=hold["opened"],
                        )
                        rbc_last = stage_b(b, expz)
                    else:
                        c_res = stage_c(
                            prev[0], prev[1], prev[2], hold["rbc"]
                        )
                    fins.append((prev[0],) + c_res)
                    if i in flush_at and len(fins) >= flush_at[i]:
                        stage_fin_group(
                            fins,
                            gpsimd_only=(i == nb - 1),
                            quarters=(i == nb - 1),
                        )
                        fins = []
                pending = (b, r_tiles.pop(i), expz)
            # drain: b7's squares all-ACT too; quarter-DMAs leave as tiles
            # normalize.
            # b7's squares split ACT/DVE: tt0/tt1 ride ACT behind w0/w1
            # while tt2/tt3 go DVE-stt right after w3 (the ACT queue showed
            # ~2us of slop reaching late squares when all four were there)
            last_c = stage_c(
                pending[0], pending[1], pending[2], rbc_last
            )
            stage_fin_last(pending[0], *last_c)

    nc.compile()
    return nc


def _pack_pmajor(a, nchunks):
    """[.., nchunks*P, S] -> [.., P, nchunks, S] partition-major contiguous."""
    lead = a.shape[:-2]
    a = a.reshape(lead + (nchunks, P, S))
    perm = tuple(range(len(lead))) + (len(lead) + 1, len(lead), len(lead) + 2)
    return np.ascontiguousarray(a.transpose(perm))


_nc_cache = None


def kernel(x1, x2, W_query, W_key, W_value, _trace=False):
    global _nc_cache, last_results
    x1T = np.asarray(x1, dtype=np.float32).transpose(0, 2, 1)
    x1r = _pack_pmajor(x1T, KT1)  # [B, P, KT1, S] f32
    x1b = _pack_pmajor(x1T.astype(BFNP), KT1)
    x2t = _pack_pmajor(
        np.asarray(x2, dtype=np.float32).transpose(0, 2, 1).astype(BFNP), KT2
    )
    wqb = _pack_pmajor(np.asarray(W_query, dtype=np.float32).astype(BFNP), KT1)
    wk = _pack_pmajor(np.asarray(W_key, dtype=np.float32).astype(BFNP), KT2)
    wv = _pack_pmajor(np.asarray(W_value, dtype=np.float32).astype(BFNP), NT)

    if _nc_cache is None:
        _nc_cache = build_nc()
    nc = _nc_cache

    in_maps = []
    for c in range(N_CORES):
        sl = slice(c * BPC, (c + 1) * BPC)
        in_maps.append(
            {
                "x1b": x1b[sl],
                "x1r": x1r[sl],
                "x2t": x2t[sl],
                "wqb": wqb,
                "wk": wk,
                "wv": wv,
            }
        )
    res = run_bass_kernel_spmd(
        nc, in_maps, core_ids=list(range(N_CORES)), trace=_trace
    )
    last_results = res
    # out: [bpc, P, NT, S] bf16 -> outT [B, S, S] -> untranspose
    outs = [np.asarray(res.results[c]["out"]) for c in range(N_CORES)]
    outT = np.concatenate(outs, axis=0).astype(np.float32)
    outT = outT.transpose(0, 2, 1, 3).reshape(B, S, S)
    return np.ascontiguousarray(outT.transpose(0, 2, 1))


# revision 22
# speedup vs baseline: 1.0890x; 1.0890x over previous
"""Trainium2 Bass kernel for nn_AttentionLayer (B=64, S=512, F=256), 8 cores.

Reference computation (per batch b):
    scores = x1 @ Wq + x2 @ Wk          # [S, S]
    a = softmax(tanh(scores), axis=-1)   # softmax over u
    a2 = a @ Wv                          # [S, S]
    out = a2 * x1                        # elementwise
    out = out * rsqrt(max(sum_s out^2, eps))   # l2-normalize over axis s

Strategy: pure data parallelism -- 8 batches per core, weights replicated.
Everything is computed in a TRANSPOSED layout ([t-or-u partitions, s free]).

Design notes (informed by HW traces; v5 of the schedule):
  * Stage A consumes a BF16 copy of x1 (all weights bf16 too), so every
    A-feed is only 0.75MB/batch and all stage-A matmuls run at bf16
    cadence (~216ns vs ~236 for f32r moving).  The f32 x1 stream needed
    by the epilogue (dtype-pure q = y*x1 on DVE) trails ~1.5 windows
    behind on the wire -- it never gates the PE.
  * The startup DMA wire saturates at ~320GB/s total across the three
    rings (sync/scalar/gpsimd); everything is emitted in consumption
    order so the critical prefix (b0 strips, then per-batch bf16 feeds)
    leads.  Batch 0 runs k-outer so strips are consumed on arrival.
  * Stage C consumes the UNNORMALIZED expz; 1/denominator folds into the
    epilogue w-pass, so no PE work waits on the rowsum->recip chain.
  * The rowsum uses a ones-BLOCK stationary so the matmul writes the
    denominator replicated across all 128 partitions.
  * The tile scheduler is greedy-by-readiness per engine (emission order
    is only a tie-break priority): the drain emits b7's rowsum/recip
    right after stage_a(b7) so the recip outranks b6's q-chain on the
    DVE the moment the rowsum lands; the PE fills the wait with C(b6).
  * Finalize groups (b0,b1),(b2,b3),(b4,b5),(b6),b7: each group's ACT
    sqrts adjacent; Square lives in BOTH activation-table sets so the
    pre-drain swap to the sqrt set also covers the drain's squares+sqrt.
  * Drain: b6+b7 squares all on ACT (idle post-exp) keeping the DVE
    clear for the q/w chains; norms split GpSimd/DVE; output leaves in
    quarter-DMAs on alternating rings as tiles normalize.
  * All DRAM tensors partition-major; output bf16, upcast on host.
"""

import sys

sys.path.insert(0, "/opt/trn_rl_repo")

import numpy as np
import ml_dtypes

import concourse.bass as bass
import concourse.tile as tile
from concourse import bacc, mybir
from concourse.bass_utils import run_bass_kernel_spmd

B, S, F = 64, 512, 256
N_CORES = 8
BPC = B // N_CORES  # batches per core
P = 128
KT1 = S // P  # 4 k-tiles over t (x1/Wq contraction)
KT2 = F // P  # 2 k-tiles over f (x2/Wk contraction)
NT = S // P  # 4 m-tiles over u (stage A) / t (stage C)
EPS = 1e-12

F32 = mybir.dt.float32
BF16 = mybir.dt.bfloat16
AF = mybir.ActivationFunctionType
ALU = mybir.AluOpType

BFNP = ml_dtypes.bfloat16

last_results = None  # test harness introspection


def build_nc(reps=1, bpc=BPC):
    nc = bacc.Bacc(
        "TRN2", target_bir_lowering=False, debug=False, num_devices=N_CORES
    )
    # Partition-major packed tensors: [.., P, ktiles, S].
    x1b = nc.declare_dram_parameter("x1b", [bpc, P, KT1, S], BF16, isOutput=False)
    x1r = nc.declare_dram_parameter("x1r", [bpc, P, KT1, S], F32, isOutput=False)
    x2t = nc.declare_dram_parameter("x2t", [bpc, P, KT2, S], BF16, isOutput=False)
    wqb = nc.declare_dram_parameter("wqb", [P, KT1, S], BF16, isOutput=False)
    wk = nc.declare_dram_parameter("wk", [P, KT2, S], BF16, isOutput=False)
    wv = nc.declare_dram_parameter("wv", [P, NT, S], BF16, isOutput=False)
    out = nc.declare_dram_parameter("out", [bpc, P, NT, S], BF16, isOutput=True)

    batches = [bb for _ in range(reps) for bb in range(bpc)]
    nb = len(batches)

    with tile.TileContext(nc) as tc:
        with (
            tc.tile_pool(name="singles", bufs=1) as singles,
            tc.tile_pool(name="xin", bufs=1) as xin,
            tc.tile_pool(name="work", bufs=2) as work,
            tc.tile_pool(name="small", bufs=2) as small,
            tc.tile_pool(name="outp", bufs=2) as outp,
            tc.tile_pool(name="psA", bufs=2, space="PSUM") as psA,
            tc.tile_pool(name="psY", bufs=3, space="PSUM") as psY,
            tc.tile_pool(name="psR", bufs=1, space="PSUM") as psR,
        ):
            ones_blk = singles.tile([P, P], BF16)
            nc.vector.memset(ones_blk, 1.0)
            eps_t = singles.tile([P, 1], F32)
            nc.vector.memset(eps_t, EPS)

            def dma_ab(b):
                """bf16 A-feed for batch b (0.75MB): x1 bf16 + x2."""
                tb = xin.tile([P, KT1, S], BF16, tag="x1b", bufs=3)
                nc.sync.dma_start(out=tb, in_=x1b.ap()[b])
                t2 = xin.tile([P, KT2, S], BF16, tag="x2", bufs=3)
                nc.gpsimd.dma_start(out=t2, in_=x2t.ap()[b])
                return tb, t2

            def dma_r(b):
                """f32 x1 for batch b's epilogue (1MB), halves on two
                rings; needed ~1.5 windows after emission."""
                tr = xin.tile([P, KT1, S], F32, tag="x1r", bufs=3)
                nc.scalar.dma_start(out=tr[:, 0:2, :], in_=x1r.ap()[b, :, 0:2, :])
                nc.sync.dma_start(out=tr[:, 2:4, :], in_=x1r.ap()[b, :, 2:4, :])
                return tr

            # Startup, in wire-consumption order: b0 strips (k-outer
            # consumes on arrival), b1's A-feed, wv, b0's f32 x1.
            wqb_t = singles.tile([P, KT1, S], BF16, tag="wqb")
            x1b0_t = xin.tile([P, KT1, S], BF16, tag="x1b", bufs=3)
            wk_t = singles.tile([P, KT2, S], BF16, tag="wk")
            x2_first = xin.tile([P, KT2, S], BF16, tag="x2", bufs=3)
            # kt0 as single-k strips so the first matmul fires ~2us sooner
            # (half-rate 1KB lines, but first-byte latency wins at the top)
            nc.sync.dma_start(out=wqb_t[:, 0:1, :], in_=wqb.ap()[:, 0:1, :])
            nc.scalar.dma_start(out=x1b0_t[:, 0:1, :], in_=x1b.ap()[0, :, 0:1, :])
            nc.sync.dma_start(out=wqb_t[:, 1:2, :], in_=wqb.ap()[:, 1:2, :])
            nc.scalar.dma_start(out=x1b0_t[:, 1:2, :], in_=x1b.ap()[0, :, 1:2, :])
            nc.gpsimd.dma_start(out=wk_t, in_=wk.ap())
            nc.sync.dma_start(out=wqb_t[:, 2:4, :], in_=wqb.ap()[:, 2:4, :])
            nc.scalar.dma_start(out=x1b0_t[:, 2:4, :], in_=x1b.ap()[0, :, 2:4, :])
            nc.gpsimd.dma_start(out=x2_first, in_=x2t.ap()[0])
            ab_tiles = {0: (x1b0_t, x2_first)}
            if nb > 1:
                ab_tiles[1] = dma_ab(1)
            wv_t = singles.tile([P, NT, S], BF16, tag="wv")
            nc.gpsimd.dma_start(out=wv_t, in_=wv.ap())
            r_tiles = {0: dma_r(0)}

            def stage_a_b0():
                """k-outer stage A for batch 0: strips consumed as they
                land; both u-pair PSUM tiles accumulate simultaneously."""
                sc01 = psA.tile([P, 2, S], F32, tag="scores")
                sc23 = psA.tile([P, 2, S], F32, tag="scores")
                scs = (sc01, sc23)
                for kt in range(KT1):
                    for half in range(2):
                        for j in range(2):
                            ut = half * 2 + j
                            us = slice(ut * P, (ut + 1) * P)
                            nc.tensor.matmul(
                                scs[half][:, j, :],
                                wqb_t[:, kt, us],
                                x1b0_t[:, kt, :],
                                start=(kt == 0),
                                stop=False,
                            )
                for kf in range(KT2):
                    for half in range(2):
                        for j in range(2):
                            ut = half * 2 + j
                            us = slice(ut * P, (ut + 1) * P)
                            nc.tensor.matmul(
                                scs[half][:, j, :],
                                wk_t[:, kf, us],
                                x2_first[:, kf, :],
                                start=False,
                                stop=(kf == KT2 - 1),
                            )
                expz = work.tile([P, NT, S], BF16, tag="expz", bufs=3)
                for half in range(2):
                    tanh_t = work.tile([P, 2, S], F32, tag="tanh")
                    nc.scalar.activation(out=tanh_t, in_=scs[half], func=AF.Tanh)
                    nc.scalar.activation(
                        out=expz[:, half * 2 : half * 2 + 2, :],
                        in_=tanh_t,
                        func=AF.Exp,
                    )
                return expz

            def stage_a(b, x1_sb, x2_sb, mid_cb=None):
                """scores matmuls in u-tile pairs sharing one 2-bank PSUM
                tile, tanh+exp over pairs.  mid_cb (if set) is emitted
                between the two pair-halves (the previous batch's rowsum
                overlaps this batch's remaining matmuls)."""
                expz = work.tile([P, NT, S], BF16, tag="expz", bufs=3)
                for half in range(NT // 2):
                    sc = psA.tile([P, 2, S], F32, tag="scores")
                    for j in range(2):
                        ut = half * 2 + j
                        us = slice(ut * P, (ut + 1) * P)
                        prods = [
                            (wqb_t[:, kt, us], x1_sb[:, kt, :]) for kt in range(KT1)
                        ] + [(wk_t[:, kt, us], x2_sb[:, kt, :]) for kt in range(KT2)]
                        for pi, (l_ap, r_ap) in enumerate(prods):
                            nc.tensor.matmul(
                                sc[:, j, :],
                                l_ap,
                                r_ap,
                                start=(pi == 0),
                                stop=(pi == len(prods) - 1),
                            )
                    tanh_t = work.tile([P, 2, S], F32, tag="tanh")
                    nc.scalar.activation(out=tanh_t, in_=sc, func=AF.Tanh)
                    nc.scalar.activation(
                        out=expz[:, half * 2 : half * 2 + 2, :],
                        in_=tanh_t,
                        func=AF.Exp,
                    )
                    if half == 0 and mid_cb is not None:
                        mid_cb()
                return expz

            def stage_b(b, expz):
                """softmax denominator (ones-block rowsum -> broadcast
                form) + reciprocal; 1/D folds into the epilogue w-pass."""
                rs = psR.tile([P, S], F32, tag="rowsum")
                for ut in range(NT):
                    nc.tensor.matmul(
                        rs,
                        ones_blk,
                        expz[:, ut, :],
                        start=(ut == 0),
                        stop=(ut == NT - 1),
                    )
                rbc = small.tile([P, S], F32, tag="rbc")
                nc.vector.reciprocal_approx_fast(out=rbc, in_=rs)
                return rbc

            def stage_c(b, x1f_sb, expz, rbc, sq_act_all=False, tiles=None,
                        opened=None):
                """Y matmuls on raw expz; epilogue q=y*x1 -> w=q*rbc (f32 on
                DVE); square-accum split ACT/DVE (or all-ACT when the DVE
                is the drain-critical chain).  `tiles` selects a subset of
                t-tiles (the drain interleaves C(b6) into A(b7) halves);
                `opened` carries (w_sb, sumsq) across split calls."""
                if opened is None:
                    w_sb = outp.tile([P, NT, S], F32, tag="w", bufs=4)
                    sumsq = small.tile([P, NT], F32, tag="sumsq", bufs=5)
                else:
                    w_sb, sumsq = opened
                for tt in (range(NT) if tiles is None else tiles):
                    y = psY.tile([P, S], F32, tag="y")
                    for ut in range(NT):
                        nc.tensor.matmul(
                            y,
                            wv_t[:, ut, tt * P : (tt + 1) * P],
                            expz[:, ut, :],
                            start=(ut == 0),
                            stop=(ut == NT - 1),
                        )
                    q_t = small.tile([P, S], F32, tag="q")
                    w_t = w_sb[:, tt, :]
                    nc.vector.tensor_tensor(
                        out=q_t, in0=y, in1=x1f_sb[:, tt, :], op=ALU.mult
                    )
                    nc.vector.tensor_tensor(out=w_t, in0=q_t, in1=rbc, op=ALU.mult)
                    if tt >= 2 and not sq_act_all:
                        scr = small.tile([P, S], F32, tag="scr")
                        nc.vector.scalar_tensor_tensor(
                            out=scr,
                            in0=w_t,
                            scalar=1.0,
                            in1=w_t,
                            op0=ALU.mult,
                            op1=ALU.mult,
                            accum_out=sumsq[:, tt : tt + 1],
                        )
                    else:
                        scr = small.tile([P, S], BF16, tag="scrb")
                        nc.scalar.activation(
                            out=scr,
                            in_=w_t,
                            func=AF.Square,
                            accum_out=sumsq[:, tt : tt + 1],
                        )
                return w_sb, sumsq

            def stage_fin_group(fins, gpsimd_only=False, quarters=False):
                """Finalize a group: ACT sqrts adjacent (one sqrt-table
                epoch per group), norms split GpSimd/DVE (all-GpSimd for
                the pre-drain flush, keeping the DVE free)."""
                rsqs = []
                for (b, w_sb, sumsq) in fins:
                    rsq = small.tile([P, NT], F32, tag="rsq", bufs=5)
                    nc.scalar.activation(
                        out=rsq, in_=sumsq, func=AF.Sqrt, bias=eps_t
                    )
                    rsqs.append(rsq)
                for (b, w_sb, sumsq), rsq in zip(fins, rsqs):
                    ob = outp.tile([P, NT, S], BF16, tag="ob", bufs=4)
                    if gpsimd_only:
                        for tt in range(NT):
                            nc.gpsimd.normalize_recip(
                                out_ap=ob[:, tt, :],
                                in_ap=w_sb[:, tt, :],
                                denom_ap=rsq[:, tt : tt + 1],
                            )
                            if quarters and tt % 2 == 1:
                                # pre-drain halves ride sync/scalar so the
                                # gpsimd ring is empty at teardown (its
                                # DRAIN otherwise spins ~5us on late DMAs)
                                q_eng = nc.sync if tt == 1 else nc.scalar
                                q_eng.dma_start(
                                    out=out.ap()[b, :, tt - 1 : tt + 1, :],
                                    in_=ob[:, tt - 1 : tt + 1, :],
                                )
                    else:
                        vv = small.tile([P, NT], F32, tag="vv", bufs=4)
                        nc.vector.reciprocal_approx_fast(out=vv, in_=rsq)
                        for tt in range(NT):
                            if tt < 2:
                                nc.gpsimd.normalize_recip(
                                    out_ap=ob[:, tt, :],
                                    in_ap=w_sb[:, tt, :],
                                    denom_ap=rsq[:, tt : tt + 1],
                                )
                            else:
                                nc.vector.tensor_scalar_mul(
                                    ob[:, tt, :],
                                    w_sb[:, tt, :],
                                    vv[:, tt : tt + 1],
                                )
                    if not (gpsimd_only and quarters):
                        nc.gpsimd.dma_start(out=out.ap()[b], in_=ob)

            def stage_fin_last(b, w_sb, sumsq):
                """Drain finalize: norms split GpSimd/DVE, quarter-DMAs on
                alternating rings as tiles complete."""
                rsq = small.tile([P, NT], F32, tag="rsq", bufs=5)
                nc.scalar.activation(out=rsq, in_=sumsq, func=AF.Sqrt, bias=eps_t)
                vv = small.tile([P, NT], F32, tag="vv", bufs=4)
                nc.vector.reciprocal_approx_fast(out=vv, in_=rsq)
                ob = outp.tile([P, NT, S], BF16, tag="ob", bufs=4)
                for tt in range(NT):
                    if tt % 2 == 0:
                        nc.gpsimd.normalize_recip(
                            out_ap=ob[:, tt, :],
                            in_ap=w_sb[:, tt, :],
                            denom_ap=rsq[:, tt : tt + 1],
                        )
                        nc.sync.dma_start(
                            out=out.ap()[b, :, tt : tt + 1, :],
                            in_=ob[:, tt : tt + 1, :],
                        )
                    else:
                        nc.vector.tensor_scalar_mul(
                            ob[:, tt, :], w_sb[:, tt, :], vv[:, tt : tt + 1]
                        )
                        nc.scalar.dma_start(
                            out=out.ap()[b, :, tt : tt + 1, :],
                            in_=ob[:, tt : tt + 1, :],
                        )

            flush_at = (
                {2: 2, 4: 2, 6: 2, nb - 1: 1}
                if nb >= 7
                else {nb - 1: nb - 1}
            )

            pending = None  # (b, x1r_tile, expz) awaiting stages B+C
            fins = []  # (b, w_sb, sumsq) awaiting finalize
            rbc_last = None
            for i, b in enumerate(batches):
                if i + 2 < nb:
                    ab_tiles[i + 2] = dma_ab(i + 2)
                if i + 1 < nb:
                    r_tiles[i + 1] = dma_r(i + 1)
                prev = pending
                hold = {}

                def mid_cb():
                    hold["rbc"] = stage_b(prev[0], prev[2])
                    if i == nb - 1:
                        # drain: interleave C(b6) INTO A(b7) -- tt0 between
                        # the halves, tt1-3 right after -- so b6's q/w DVE
                        # chain starts ~4us before the PE finishes.  b6's
                        # squares all ride ACT (idle post-exp) keeping the
                        # DVE clear.  PSUM peak stays at 7/8 banks.
                        hold["opened"] = stage_c(
                            prev[0], prev[1], prev[2], hold["rbc"],
                            sq_act_all=True, tiles=[0],
                        )

                if i == 0:
                    expz = stage_a_b0()
                else:
                    xb, x2c = ab_tiles.pop(i)
                    expz = stage_a(b, xb, x2c, mid_cb)
                if prev is not None:
                    if i == nb - 1:
                        c_res = stage_c(
                            prev[0], prev[1], prev[2], hold["rbc"],
                            sq_act_all=True, tiles=[1, 2, 3],
                            opened=hold["opened"],
                        )
                        rbc_last = stage_b(b, expz)
                    else:
                        c_res = stage_c(
                            prev[0], prev[1], prev[2], hold["rbc"]
                        )
                    fins.append((prev[0],) + c_res)
                    if i in flush_at and len(fins) >= flush_at[i]:
                        stage_fin_group(
                            fins,
                            gpsimd_only=(i == nb - 1),
                            quarters=(i == nb - 1),
                        )
                        fins = []
                pending = (b, r_tiles.pop(i), expz)
            # drain: b7's squares all-ACT too; quarter-DMAs leave as tiles
            # normalize.
            # b7's squares split ACT/DVE: tt0/tt1 ride ACT behind w0/w1
            # while tt2/tt3 go DVE-stt right after w3 (the ACT queue showed
            # ~2us of slop reaching late squares when all four were there)
            last_c = stage_c(
                pending[0], pending[1], pending[2], rbc_last
            )
            stage_fin_last(pending[0], *last_c)

    nc.compile()
    return nc


def _pack_pmajor(a, nchunks):
    """[.., nchunks*P, S] -> [.., P, nchunks, S] partition-major contiguous."""
    lead = a.shape[:-2]
    a = a.reshape(lead + (nchunks, P, S))
    perm = tuple(range(len(lead))) + (len(lead) + 1, len(lead), len(lead) + 2)
    return np.ascontiguousarray(a.transpose(perm))


_nc_cache = None


def kernel(x1, x2, W_query, W_key, W_value, _trace=False):
    global _nc_cache, last_results
    x1T = np.asarray(x1, dtype=np.float32).transpose(0, 2, 1)
    x1r = _pack_pmajor(x1T, KT1)  # [B, P, KT1, S] f32
    x1b = _pack_pmajor(x1T.astype(BFNP), KT1)
    x2t = _pack_pmajor(
        np.asarray(x2, dtype=np.float32).transpose(0, 2, 1).astype(BFNP), KT2
    )
    wqb = _pack_pmajor(np.asarray(W_query, dtype=np.float32).astype(BFNP), KT1)
    wk = _pack_pmajor(np.asarray(W_key, dtype=np.float32).astype(BFNP), KT2)
    wv = _pack_pmajor(np.asarray(W_value, dtype=np.float32).astype(BFNP), NT)

    if _nc_cache is None:
        _nc_cache = build_nc()
    nc = _nc_cache

    in_maps = []
    for c in range(N_CORES):
        sl = slice(c * BPC, (c + 1) * BPC)
        in_maps.append(
            {
                "x1b": x1b[sl],
                "x1r": x1r[sl],
                "x2t": x2t[sl],
                "wqb": wqb,
                "wk": wk,
                "wv": wv,
            }
        )
    res = run_bass_kernel_spmd(
        nc, in_maps, core_ids=list(range(N_CORES)), trace=_trace
    )
    last_results = res
    # out: [bpc, P, NT, S] bf16 -> outT [B, S, S] -> untranspose
    outs = [np.asarray(res.results[c]["out"]) for c in range(N_CORES)]
    outT = np.concatenate(outs, axis=0).astype(np.float32)
    outT = outT.transpose(0, 2, 1, 3).reshape(B, S, S)
    return np.ascontiguousarray(outT.transpose(0, 2, 1))
